# revision 6
# baseline (speedup 1.0000x reference)
"""MeshEdgeBlockConcat kernel for 8 Trainium2 NeuronCores.

Reference computation (per edge e):
    cat   = [efeat[e], nfeat[src[e]], nfeat[dst[e]]]          # [384]
    h     = silu(cat @ w1 + b1)                               # [128]
    y     = h @ w2 + b2                                       # [128]
    y     = LN(y) * gamma + beta
    out_e = y + efeat[e]
Returns (efeat_new, nfeat).

Strategy: shard edges across the 8 cores; replicate nfeat + weights.
Per 512-edge block: stream efeat + indices, gather nfeat rows with
indirect DMA (128 rows / instruction), transpose the three concat parts
to feature-major with the PE, run the two matmuls in fp32, LayerNorm in
edge-major layout with bn_stats/bn_aggr, add the residual, store.
"""
import contextlib
import ctypes
import os
import sys
import types

import numpy as np

import concourse.bass as bass
import concourse.tile as tile
from concourse import mybir
from concourse.bass_utils import run_bass_kernel_spmd
from concourse.masks import make_identity

last_exec_time_ns = None


def _install_ntff_hook():
    """Register the antenv.axon_hooks module this image is missing so
    run_bass_kernel_spmd(trace=True) can capture NTFF profiles."""
    if "antenv.axon_hooks" in sys.modules:
        return
    try:
        import antenv

        lib = ctypes.CDLL("/opt/axon/libaxon_pjrt.so")
        if not hasattr(lib, "axon_start_nrt_profile"):
            return
        lib.axon_start_nrt_profile.argtypes = [ctypes.POINTER(ctypes.c_int64), ctypes.c_size_t]
        lib.axon_start_nrt_profile.restype = ctypes.c_int64
        lib.axon_stop_nrt_profile.argtypes = [ctypes.c_char_p]
        lib.axon_stop_nrt_profile.restype = ctypes.c_int64

        @contextlib.contextmanager
        def _hook(output_dir, device_ids):
            import jax

            jax.devices()
            if device_ids:
                ids = (ctypes.c_int64 * len(device_ids))(*device_ids)
                rc = lib.axon_start_nrt_profile(ids, len(device_ids))
            else:
                rc = lib.axon_start_nrt_profile(None, 0)
            if rc != 0:
                raise RuntimeError(f"axon_start_nrt_profile rc={rc}")
            try:
                yield
            finally:
                lib.axon_stop_nrt_profile(str(output_dir).encode())

        mod = types.ModuleType("antenv.axon_hooks")
        mod.get_axon_ntff_profile_hook = lambda: _hook
        mod.set_axon_ntff_profile_hook = lambda h: None
        sys.modules["antenv.axon_hooks"] = mod
        antenv.axon_hooks = mod
    except Exception:
        pass

E, N, D, H = 500000, 100000, 128, 128
NCORES = 8
P = 128
B = 512            # edges per block
J = B // P         # 4 edge-groups per block
LN_EPS = 1e-5

F32 = mybir.dt.float32
I32 = mybir.dt.int32


# ---------------------------------------------------------------- birpatch --
def _split_multiwait(nc, max_waits=1):
    """walrus in this env accepts only one sync-wait per instruction; hoist
    extra waits emitted by Tile onto dedicated NoOps just before."""
    uid = 0
    for fn in nc.m.functions:
        for b in fn.blocks:
            lst = b.instructions
            i = 0
            while i < len(lst):
                ins = lst[i]
                si = ins.sync_info
                if si is not None and len(si.on_wait) > max_waits:
                    waits = list(si.on_wait)
                    keep = waits[-max_waits:]
                    extra = waits[:-max_waits]
                    for j, w in enumerate(extra):
                        nop = mybir.InstNoOp(name=f"WSPLIT-{uid}")
                        uid += 1
                        nop.engine = ins.engine
                        nop.sync_info = mybir.SyncInfo(on_wait=[w], on_update=[])
                        lst.insert(i + j, nop)
                    si.on_wait = keep
                    i += len(extra)
                i += 1


# ------------------------------------------------------------------- build --
def build_nc(n_blocks, n_nodes):
    epc = n_blocks * B  # padded edges per core
    nc = bass.Bass()
    efeat_d = nc.dram_tensor("efeat", [epc, D], F32, kind="ExternalInput")
    nfeat_d = nc.dram_tensor("nfeat", [n_nodes, D], F32, kind="ExternalInput")
    src_d = nc.dram_tensor("src", [epc], I32, kind="ExternalInput")
    dst_d = nc.dram_tensor("dst", [epc], I32, kind="ExternalInput")
    w1_d = nc.dram_tensor("w1", [3 * D, H], F32, kind="ExternalInput")
    b1_d = nc.dram_tensor("b1", [H], F32, kind="ExternalInput")
    w2_d = nc.dram_tensor("w2", [H, D], F32, kind="ExternalInput")
    b2_d = nc.dram_tensor("b2", [D], F32, kind="ExternalInput")
    gamma_d = nc.dram_tensor("gamma", [D], F32, kind="ExternalInput")
    beta_d = nc.dram_tensor("beta", [D], F32, kind="ExternalInput")
    out_d = nc.dram_tensor("out", [epc, D], F32, kind="ExternalOutput")

    ef_r = efeat_d.ap().rearrange("(b p j) d -> b p j d", p=P, j=J)
    out_r = out_d.ap().rearrange("(b p j) d -> b p j d", p=P, j=J)
    src_r = src_d.ap().rearrange("(b p j) -> b p j", p=P, j=J)
    dst_r = dst_d.ap().rearrange("(b p j) -> b p j", p=P, j=J)

    def bcast_row(dram_t):
        # [D] dram vector -> [P, J, D] AP replicated over partitions + groups
        return bass.AP(tensor=dram_t, offset=0, ap=[[0, P], [0, J], [1, D]])

    with tile.TileContext(nc) as tc:
        with (
            tc.tile_pool(name="const", bufs=1) as cp,
            tc.tile_pool(name="io", bufs=3) as iop,
            tc.tile_pool(name="gat", bufs=3) as gp,
            tc.tile_pool(name="mid", bufs=3) as mp,
            tc.tile_pool(name="small", bufs=6) as sp,
            tc.tile_pool(name="tp_ps", bufs=3, space="PSUM") as tpp,
            tc.tile_pool(name="h_ps", bufs=2, space="PSUM") as hpp,
            tc.tile_pool(name="y_ps", bufs=2, space="PSUM") as ypp,
        ):
            ident = cp.tile([P, P], F32)
            make_identity(nc, ident[:])
            w1k = [cp.tile([P, H], F32, tag=f"w1_{k}", name=f"w1k{k}") for k in range(3)]
            for k in range(3):
                nc.sync.dma_start(out=w1k[k][:], in_=w1_d.ap()[k * D:(k + 1) * D, :])
            w2t = cp.tile([H, D], F32)
            nc.sync.dma_start(out=w2t[:], in_=w2_d.ap()[:, :])
            b1t = cp.tile([P, 1], F32)
            nc.sync.dma_start(out=b1t[:], in_=b1_d.ap()[:, None])
            b2bc = cp.tile([P, J, D], F32, tag="b2bc")
            nc.gpsimd.dma_start(out=b2bc[:], in_=bcast_row(b2_d))
            gbc = cp.tile([P, J, D], F32, tag="gbc")
            nc.gpsimd.dma_start(out=gbc[:], in_=bcast_row(gamma_d))
            bbc = cp.tile([P, J, D], F32, tag="bbc")
            nc.gpsimd.dma_start(out=bbc[:], in_=bcast_row(beta_d))
            epst = cp.tile([P, 1], F32)
            nc.vector.memset(epst[:], LN_EPS)

            for bi in range(n_blocks):
                et = iop.tile([P, J, D], F32, tag="et")
                nc.sync.dma_start(out=et[:], in_=ef_r[bi])
                sidx = sp.tile([P, J], I32, tag="sidx")
                nc.sync.dma_start(out=sidx[:], in_=src_r[bi])
                didx = sp.tile([P, J], I32, tag="didx")
                nc.sync.dma_start(out=didx[:], in_=dst_r[bi])

                gs = [gp.tile([P, D], F32, tag=f"gs{j}", name=f"gs{j}") for j in range(J)]
                gd = [gp.tile([P, D], F32, tag=f"gd{j}", name=f"gd{j}") for j in range(J)]
                for j in range(J):
                    nc.gpsimd.indirect_dma_start(
                        out=gs[j][:], out_offset=None, in_=nfeat_d.ap(),
                        in_offset=bass.IndirectOffsetOnAxis(ap=sidx[:, j:j + 1], axis=0),
                    )
                    nc.gpsimd.indirect_dma_start(
                        out=gd[j][:], out_offset=None, in_=nfeat_d.ap(),
                        in_offset=bass.IndirectOffsetOnAxis(ap=didx[:, j:j + 1], axis=0),
                    )

                # transpose the three cat parts to feature-major [128f, B]
                catT = []
                for part in range(3):
                    tp = tpp.tile([P, J, P], F32, tag="tp")
                    for j in range(J):
                        in_t = et[:, j, :] if part == 0 else (gs[j][:] if part == 1 else gd[j][:])
                        nc.tensor.transpose(out=tp[:, j, :], in_=in_t, identity=ident[:])
                    ct = mp.tile([P, J, P], F32, tag=f"catT{part}", name=f"catT{part}")
                    nc.scalar.copy(out=ct[:], in_=tp[:])
                    catT.append(ct)

                h_ps = hpp.tile([P, J, P], F32, tag="h_ps")
                for k in range(3):
                    nc.tensor.matmul(
                        out=h_ps[:], lhsT=w1k[k][:], rhs=catT[k][:],
                        start=(k == 0), stop=(k == 2),
                    )
                h_sb = mp.tile([P, J, P], F32, tag="h_sb")
                nc.scalar.activation(
                    out=h_sb[:], in_=h_ps[:],
                    func=mybir.ActivationFunctionType.Silu,
                    bias=b1t[:], scale=1.0,
                )

                y_ps = ypp.tile([P, J, P], F32, tag="y_ps")
                for j in range(J):
                    nc.tensor.matmul(
                        out=y_ps[:, j, :], lhsT=h_sb[:, j, :], rhs=w2t[:],
                        start=True, stop=True,
                    )
                y_sb = mp.tile([P, J, D], F32, tag="y_sb")
                nc.vector.tensor_add(out=y_sb[:], in0=y_ps[:], in1=b2bc[:])

                for j in range(J):
                    stats = sp.tile([P, 6], F32, tag="stats")
                    nc.vector.bn_stats(out=stats[:], in_=y_sb[:, j, :])
                    mv = sp.tile([P, 2], F32, tag="mv")
                    nc.vector.bn_aggr(out=mv[:], in_=stats[:])
                    nc.scalar.activation(
                        out=mv[:, 1:2], in_=mv[:, 1:2],
                        func=mybir.ActivationFunctionType.Sqrt,
                        bias=epst[:], scale=1.0,
                    )
                    nc.vector.reciprocal(out=mv[:, 1:2], in_=mv[:, 1:2])
                    nc.vector.tensor_scalar(
                        out=y_sb[:, j, :], in0=y_sb[:, j, :],
                        scalar1=mv[:, 0:1], scalar2=mv[:, 1:2],
                        op0=mybir.AluOpType.subtract, op1=mybir.AluOpType.mult,
                    )

                ot = iop.tile([P, J, D], F32, tag="ot")
                nc.vector.tensor_mul(out=ot[:], in0=y_sb[:], in1=gbc[:])
                nc.vector.tensor_add(out=et[:], in0=et[:], in1=bbc[:])
                nc.vector.tensor_add(out=ot[:], in0=ot[:], in1=et[:])
                nc.sync.dma_start(out=out_r[bi], in_=ot[:])

    return nc


# -------------------------------------------------------------------- host --
def kernel(efeat, nfeat, src, dst, w1, b1, w2, b2, gamma, beta):
    e_total = efeat.shape[0]
    n_nodes = nfeat.shape[0]
    n_blocks = -(-e_total // (NCORES * B))  # ceil
    epc = n_blocks * B
    e_pad = NCORES * epc

    efeat_p = np.zeros((e_pad, D), np.float32)
    efeat_p[:e_total] = np.asarray(efeat, np.float32)
    src_p = np.zeros((e_pad,), np.int32)
    src_p[:e_total] = np.asarray(src).astype(np.int32)
    dst_p = np.zeros((e_pad,), np.int32)
    dst_p[:e_total] = np.asarray(dst).astype(np.int32)

    nfeat = np.ascontiguousarray(np.asarray(nfeat, np.float32))
    w1 = np.ascontiguousarray(np.asarray(w1, np.float32))
    b1 = np.ascontiguousarray(np.asarray(b1, np.float32))
    w2 = np.ascontiguousarray(np.asarray(w2, np.float32))
    b2 = np.ascontiguousarray(np.asarray(b2, np.float32))
    gamma = np.ascontiguousarray(np.asarray(gamma, np.float32))
    beta = np.ascontiguousarray(np.asarray(beta, np.float32))

    nc = build_nc(n_blocks, n_nodes)
    _split_multiwait(nc)
    in_maps = []
    for c in range(NCORES):
        sl = slice(c * epc, (c + 1) * epc)
        in_maps.append({
            "efeat": efeat_p[sl], "nfeat": nfeat,
            "src": src_p[sl], "dst": dst_p[sl],
            "w1": w1, "b1": b1, "w2": w2, "b2": b2,
            "gamma": gamma, "beta": beta,
        })
    trace = bool(os.environ.get("KERNEL_TRACE"))
    if trace:
        _install_ntff_hook()
    res = run_bass_kernel_spmd(nc, in_maps, list(range(NCORES)), trace=trace)
    global last_exec_time_ns
    last_exec_time_ns = res.exec_time_ns
    out = np.concatenate([res.results[c]["out"] for c in range(NCORES)], axis=0)
    return out[:e_total], nfeat
